# revision 1
# baseline (speedup 1.0000x reference)
"""Trainium2 Bass kernel for a KAN layer (piecewise-cubic spline edges).

y[b, j] = scale[j] * sum_i sum_p coeff[j, i, seg(x[b,i]), p] * t(x[b,i])^p

with 9 uniform segments on [-1, 1], t the within-segment coordinate.

Strategy:
  * Recast as one-hot-masked GEMM: y^T[j, b] = sum_{s,p,ichunk}
        coeffT[s,p,ichunk,:,j]^T @ (mask_s * t^p)[ichunk,:,b]
  * 8-way data parallel over batch (each core: 512 batch cols, full OUT).
  * Masked-power tiles built on DVE/ACT/GPSIMD, matmuls in float32r
    (fp32 stored, fp22 multiply, fp32 accumulate) at full PE rate.
"""

import numpy as np

import concourse.bass as bass
import concourse.mybir as mybir
from concourse import bacc
from concourse.tile import TileContext
from concourse.bass_utils import run_bass_kernel_spmd

AF = mybir.ActivationFunctionType
OP = mybir.AluOpType
F32 = mybir.dt.float32
F32R = mybir.dt.float32r

B, IN, OUT = 4096, 512, 512
S, P = 9, 4            # segments, polynomial terms
NC = 8                 # cores
NB = B // NC           # local batch (moving free dim)
ICH = IN // 128        # input chunks (contraction tiles)
JT = OUT // 128        # output-row tiles
UMAX = float(np.nextafter(np.float32(9.0), np.float32(0.0)))

# Tunables
AT_BUFS = 6            # in-flight masked-power tile groups
CT_BUFS = 4            # in-flight coeff tile groups
MT3_ON_GPSIMD = False  # build t^3 tiles on GPSIMD (else VectorE)
DMA_BEFORE_AT = False  # emit coeff DMA before masked-power ops
JT_OUTER = True        # matmul inner loops: jt outer / p inner

LAST_EXEC_NS = None
LAST_RESULTS = None
LAST_NC = None
LAST_IN_MAPS = None


def _build_nc():
    nc = bacc.Bacc("TRN2", target_bir_lowering=False, debug=False, num_devices=NC)

    xt_d = nc.dram_tensor("xt", [IN, NB], F32, kind="ExternalInput")
    cf_d = nc.dram_tensor("coeffr", [S * ICH, 128, P * JT * 128], F32R,
                          kind="ExternalInput")
    sc_d = nc.dram_tensor("scale", [OUT, 1], F32, kind="ExternalInput")
    yt_d = nc.dram_tensor("yt", [OUT, NB], F32, kind="ExternalOutput")

    with TileContext(nc) as tc:
        with (
            tc.tile_pool(name="xp", bufs=1) as xp,
            tc.tile_pool(name="atp", bufs=AT_BUFS) as atp,
            tc.tile_pool(name="ctp", bufs=CT_BUFS) as ctp,
            tc.tile_pool(name="outp", bufs=2) as outp,
            tc.tile_pool(name="pp", bufs=1, space="PSUM") as pp,
        ):
            xt_sb = xp.tile([128, ICH, NB], F32, name="xt_sb")
            xt_r = xt_d.rearrange("(c p) b -> p c b", p=128)
            for ic in range(ICH):
                nc.sync.dma_start(xt_sb[:, ic], xt_r[:, ic])
            sc_sb = xp.tile([128, JT, 1], F32, name="sc_sb")
            nc.sync.dma_start(sc_sb, sc_d.rearrange("(c p) o -> p c o", p=128))

            u_sb = xp.tile([128, ICH, NB], F32, name="u_sb")
            t_sb = xp.tile([128, ICH, NB], F32, name="t_sb")
            seg_sb = xp.tile([128, ICH, NB], F32, name="seg_sb")
            segi_sb = xp.tile([128, ICH, NB], mybir.dt.int32, name="segi_sb")

            for ic in range(ICH):
                xs = xt_sb[:, ic]
                us = u_sb[:, ic]
                ts = t_sb[:, ic]
                ss = seg_sb[:, ic]
                # u2 = clip(x,-1,1)*4.5 + 4.0 in [-0.5, 8.5]; RNE(u2) == floor
                # of the segment coordinate (verified exact vs searchsorted).
                nc.vector.tensor_scalar(us, xs, 1.0, -1.0, OP.min, OP.max)
                nc.vector.tensor_scalar(us, us, 4.5, 4.0, OP.mult, OP.add)
                nc.vector.tensor_copy(segi_sb[:, ic], us)            # RNE -> int32
                nc.vector.tensor_copy(ss, segi_sb[:, ic])            # back to f32
                # t = (u2 + 0.5) - seg
                nc.vector.scalar_tensor_tensor(ts, us, 0.5, ss, OP.add, OP.subtract)

            ps = [pp.tile([128, NB], F32, name=f"ps{jt}", tag=f"ps{jt}")
                  for jt in range(JT)]

            for s in range(S):
                for ic in range(ICH):
                    at = atp.tile([128, P, NB], F32R, name=f"at_{s}_{ic}", tag="at")
                    ct = ctp.tile([128, JT, P * 128], F32R, name=f"ct_{s}_{ic}",
                                  tag="ct")
                    cf_g = cf_d[s * ICH + ic].rearrange("p (j q) -> p j q", j=JT)
                    if DMA_BEFORE_AT:
                        nc.sync.dma_start(ct, cf_g)
                    ts = t_sb[:, ic]
                    nc.vector.tensor_scalar(at[:, 0], seg_sb[:, ic], float(s), None,
                                            OP.is_equal)
                    nc.vector.tensor_mul(at[:, 1], at[:, 0], ts)
                    nc.scalar.activation(at[:, 2], at[:, 1], AF.Square)
                    eng3 = nc.gpsimd if MT3_ON_GPSIMD else nc.vector
                    eng3.tensor_mul(at[:, 3], at[:, 2], at[:, 1])
                    if not DMA_BEFORE_AT:
                        nc.sync.dma_start(ct, cf_g)

                    first = (s == 0 and ic == 0)
                    last = (s == S - 1 and ic == ICH - 1)
                    pjt = ([(p, jt) for jt in range(JT) for p in range(P)]
                           if JT_OUTER else
                           [(p, jt) for p in range(P) for jt in range(JT)])
                    for p, jt in pjt:
                        nc.tensor.matmul(
                            ps[jt][:, :],
                            lhsT=ct[:, jt, p * 128:(p + 1) * 128],
                            rhs=at[:, p, :],
                            start=(first and p == 0),
                            stop=(last and p == P - 1),
                        )

            for jt in range(JT):
                ot = outp.tile([128, NB], F32, name=f"ot{jt}", tag="ot")
                nc.scalar.activation(ot, ps[jt], AF.Copy, scale=sc_sb[:, jt])
                nc.sync.dma_start(yt_d[jt * 128:(jt + 1) * 128, :], ot)

    nc.compile()
    return nc


def kernel(x, coeff, scale, _trace=False):
    global LAST_EXEC_NS, LAST_RESULTS, LAST_NC, LAST_IN_MAPS
    x = np.ascontiguousarray(np.asarray(x, dtype=np.float32))
    coeff = np.ascontiguousarray(np.asarray(coeff, dtype=np.float32))
    scale = np.ascontiguousarray(np.asarray(scale, dtype=np.float32))

    # x^T shards: [IN, NB] per core
    xt = np.ascontiguousarray(x.T)
    # Round coeff to fp22 (e8m13, what the PE multiplies in) with RNE on the
    # host so the on-device f32r truncation is lossless.
    cb = coeff.view(np.uint32)
    cb = (cb + np.uint32(0x1FF) + ((cb >> np.uint32(10)) & np.uint32(1))) & \
        np.uint32(0xFFFFFC00)
    coeff = cb.view(np.float32)
    # coeff [OUT, IN, S, P] -> tiles [(s, ic), i_in, (p, jt, j_in)]
    cr = coeff.transpose(2, 3, 1, 0)                      # [S, P, IN, OUT]
    cr = cr.reshape(S, P, ICH, 128, JT, 128)              # s p ic i_in jt j_in
    cr = cr.transpose(0, 2, 3, 4, 1, 5)                   # s ic i_in jt p j_in
    cr = np.ascontiguousarray(cr.reshape(S * ICH, 128, P * JT * 128))
    sc2 = scale.reshape(OUT, 1)

    nc = _build_nc()
    in_maps = [
        {"xt": np.ascontiguousarray(xt[:, g * NB:(g + 1) * NB]),
         "coeffr": cr, "scale": sc2}
        for g in range(NC)
    ]
    res = run_bass_kernel_spmd(nc, in_maps, core_ids=list(range(NC)),
                               trace=_trace)
    LAST_RESULTS = res
    LAST_EXEC_NS = res.exec_time_ns
    LAST_NC = nc
    LAST_IN_MAPS = in_maps

    yt = np.concatenate([res.results[g]["yt"] for g in range(NC)], axis=1)
    return np.ascontiguousarray(yt.T)



# revision 23
# speedup vs baseline: 2.3552x; 2.3552x over previous
"""Trainium2 Bass kernel for a KAN layer (piecewise-cubic spline edges).

y[b, j] = scale[j] * sum_i sum_p coeff[j, i, seg(x[b,i]), p] * t(x[b,i])^p

with 9 uniform segments on [-1, 1], t the within-segment coordinate.

Strategy (fp8e4m3 + DoubleRow, 6 coefficient slots):
  * One-hot-masked GEMM: all matmul operands are fp8e4m3 and every matmul
    uses MatmulPerfMode.DoubleRow (256 contraction rows/instruction at 0.5
    cycles per output element).  3 DoubleRow matmuls per (segment, ichunk,
    jtile): hi(p0,p1), hi(p2,p3), lo(l0,l1).
  * Coefficient slots: hi = Q8(c*scale) for all 4 powers; the lo pair rides
    the (mask, mask*t) moving tiles and carries the least-squares projection
    of the full quantization residual onto span{1, t} (per-segment moments
    of the clipped-normal t-distribution) -> rel err ~1.2e-2, same as an
    8-slot hi/lo split, at 3/4 the PE work.
  * Moving tiles via byte tricks: seg as int8; m01 = (seg==s) per byte;
    mFF = m01 * 255 (int32 view; (m<<8)-m has no cross-byte borrow);
    at0/at1/at3 = bitwise-AND selects of fp8 t-power bytes (DVE int32 ops,
    2x mode); at2 = ACT Square(at1) since (m*t)^2 = m*t^2.  GPSIMD covers
    t^3 products.  All engines stay under the PE's 46us of matmul work.
  * 8-way data parallel over batch (each core: 512 batch cols, full OUT).
"""

import numpy as np
import ml_dtypes

import concourse.bass as bass
import concourse.mybir as mybir
from concourse import bacc
from concourse.tile import TileContext
from concourse.bass_utils import run_bass_kernel_spmd

AF = mybir.ActivationFunctionType
OP = mybir.AluOpType
PM = mybir.MatmulPerfMode
F32 = mybir.dt.float32
FP8 = mybir.dt.float8e4
I32 = mybir.dt.int32
I8 = mybir.dt.int8
NP8 = ml_dtypes.float8_e4m3

B, IN, OUT = 4096, 512, 512
S, P = 9, 4            # segments, polynomial terms
NC = 8                 # cores
NB = B // NC           # local batch (moving free dim)
ICH = IN // 128        # input chunks (contraction tiles)
JT = OUT // 128        # output-row tiles
NSLOT = 6              # coeff pair slots: hi p0..p3, lo l0..l1

# Tunables
AT_BUFS = 8            # in-flight masked-power tile groups
CT_BUFS = 8            # in-flight coeff tile groups
N_WARM = 30            # PE p-state warm-up dummy matmuls before the stream
# (k, weight-slot pair, use at[0:2]) order: at01-consumers first
K_PLAN = ((0, 0, True), (2, 4, True), (1, 2, False))

LAST_EXEC_NS = None
LAST_RESULTS = None
LAST_NC = None
LAST_IN_MAPS = None


def _build_nc():
    nc = bacc.Bacc("TRN2", target_bir_lowering=False, debug=False, num_devices=NC)

    xt_d = nc.dram_tensor("xt", [IN, NB], F32, kind="ExternalInput")
    cf_d = nc.dram_tensor("coeff8", [ICH * S, 128, NSLOT * JT * 128], FP8,
                          kind="ExternalInput")
    yt_d = nc.dram_tensor("yt", [OUT, NB], F32, kind="ExternalOutput")

    with TileContext(nc) as tc:
        with (
            tc.tile_pool(name="xp", bufs=1) as xp,
            tc.tile_pool(name="mp", bufs=4) as mp,
            tc.tile_pool(name="atp", bufs=AT_BUFS) as atp,
            tc.tile_pool(name="ctp", bufs=CT_BUFS) as ctp,
            tc.tile_pool(name="outp", bufs=JT) as outp,
            tc.tile_pool(name="pp", bufs=1, space="PSUM") as pp,
        ):
            xt_sb = xp.tile([128, ICH, NB], F32, name="xt_sb")
            xt_r = xt_d.rearrange("(c p) b -> p c b", p=128)

            # PE p-state warm-up: harmless DoubleRow matmuls on a zeroed tile
            # into a scratch PSUM bank keep the tensor engine continuously
            # busy from ~1.4us so the real stream starts at full clock.
            zt = xp.tile([128, 2, NB], FP8, name="zt")
            psd = pp.tile([128, NB], F32, name="psd", tag="psd")
            nc.vector.memset(zt, 0.0)
            for _ in range(N_WARM):
                nc.tensor.matmul(psd, lhsT=zt[:, :, 0:128], rhs=zt,
                                 start=True, stop=True, perf_mode=PM.DoubleRow)

            # x for ic0 first, then the first coeff tile, then the rest of x,
            # so the head-of-pipe dependencies resolve earliest.
            nc.sync.dma_start(xt_sb[:, 0], xt_r[:, 0])
            ct0 = ctp.tile([128, NSLOT, JT * 128], FP8, name="ct_0_0", tag="ct")
            nc.sync.dma_start(ct0, cf_d[0].rearrange("p (k q) -> p k q", k=NSLOT))
            for ic in range(1, ICH):
                nc.sync.dma_start(xt_sb[:, ic], xt_r[:, ic])

            xc_sb = xp.tile([128, ICH, NB], F32, name="xc_sb")
            t_sb = xp.tile([128, ICH, NB], F32, name="t_sb")
            t2_sb = xp.tile([128, ICH, NB], F32, name="t2_sb")
            t3_sb = xp.tile([128, ICH, NB], F32, name="t3_sb")
            seg_sb = xp.tile([128, ICH, NB], F32, name="seg_sb")  # seg - 4.5
            segb_sb = xp.tile([128, ICH, NB], I8, name="segb_sb")
            t8_sb = xp.tile([128, ICH, NB], FP8, name="t8_sb")
            t28_sb = xp.tile([128, ICH, NB], FP8, name="t28_sb")
            t38_sb = xp.tile([128, ICH, NB], FP8, name="t38_sb")

            ps = [pp.tile([128, NB], F32, name=f"ps{jt}", tag=f"ps{jt}")
                  for jt in range(JT)]

            for ic in range(ICH):
                xs = xt_sb[:, ic]
                xcs = xc_sb[:, ic]
                ts = t_sb[:, ic]
                t2s = t2_sb[:, ic]
                t3s = t3_sb[:, ic]
                ss = seg_sb[:, ic]
                sgb = segb_sb[:, ic]
                # xc = clip(x,-1,1); seg byte = RNE(xc*4.5+4.0) (== floor of
                # the segment coordinate, verified exact vs searchsorted);
                # ss = seg - 4.5; t = 4.5*xc - ss
                nc.vector.tensor_scalar(xcs, xs, 1.0, -1.0, OP.min, OP.max)
                nc.vector.tensor_scalar(sgb, xcs, 4.5, 4.0, OP.mult, OP.add)
                nc.vector.tensor_scalar(ss, sgb, -4.5, None, OP.add)
                nc.vector.scalar_tensor_tensor(ts, xcs, 4.5, ss,
                                               OP.mult, OP.subtract)
                # fp8 t powers for AND-selects; f32 t^2, t^3 for Pool products
                nc.scalar.activation(t8_sb[:, ic], ts, AF.Copy)
                nc.scalar.activation(t2s, ts, AF.Square)
                nc.scalar.activation(t28_sb[:, ic], t2s, AF.Copy)
                nc.gpsimd.tensor_tensor(t3s, t2s, ts, OP.mult)
                nc.scalar.activation(t38_sb[:, ic], t3s, AF.Copy)

                for s in range(S):
                    at = atp.tile([128, P, NB], FP8, name=f"at_{ic}_{s}", tag="at")
                    if ic == 0 and s == 0:
                        ct = ct0
                    else:
                        ct = ctp.tile([128, NSLOT, JT * 128], FP8,
                                      name=f"ct_{ic}_{s}", tag="ct")
                        nc.sync.dma_start(ct, cf_d[ic * S + s].rearrange(
                            "p (k q) -> p k q", k=NSLOT))
                    # byte mask 0xFF where seg==s in ONE dual op, then
                    # bitwise AND-selects of the fp8 t-power bytes (int32
                    # views, DVE 2x path)
                    mff = mp.tile([128, NB], I8, name=f"mff_{ic}_{s}",
                                  tag="mff")
                    nc.vector.tensor_scalar(mff, sgb, s, -1,
                                            OP.is_equal, OP.mult)
                    nc.vector.tensor_scalar(at[:, 0].bitcast(I32),
                                            mff.bitcast(I32),
                                            0x38383838, None, OP.bitwise_and)
                    nc.vector.tensor_tensor(at[:, 1].bitcast(I32),
                                            t8_sb[:, ic].bitcast(I32),
                                            mff.bitcast(I32), OP.bitwise_and)
                    if ic == 0 and s == 0:
                        # head: t28/t38 aren't ready yet; stt from f32 powers
                        nc.vector.scalar_tensor_tensor(
                            at[:, 2], ss, float(s) - 4.5, t2s,
                            OP.is_equal, OP.mult)
                        nc.vector.scalar_tensor_tensor(
                            at[:, 3], ss, float(s) - 4.5, t3s,
                            OP.is_equal, OP.mult)
                    else:
                        nc.vector.tensor_tensor(at[:, 2].bitcast(I32),
                                                t28_sb[:, ic].bitcast(I32),
                                                mff.bitcast(I32),
                                                OP.bitwise_and)
                        if s % 3 == 0:
                            nc.vector.tensor_tensor(
                                at[:, 3].bitcast(I32),
                                t38_sb[:, ic].bitcast(I32),
                                mff.bitcast(I32), OP.bitwise_and)
                        else:
                            nc.gpsimd.tensor_tensor(at[:, 3], at[:, 0], t3s,
                                                    OP.mult)

                    first = (ic == 0 and s == 0)
                    last = (ic == ICH - 1 and s == S - 1)
                    ctj = ct.rearrange("p k (j q) -> p k j q", j=JT)
                    if not last:
                        for k, w0, use01 in K_PLAN:
                            rhs = at[:, 0:2] if use01 else at[:, 2:4]
                            for jt in range(JT):
                                nc.tensor.matmul(
                                    ps[jt][:, :],
                                    lhsT=ctj[:, w0:w0 + 2, jt],
                                    rhs=rhs,
                                    start=(first and k == 0),
                                    stop=False,
                                    perf_mode=PM.DoubleRow,
                                )
                    else:
                        # jt-major so the per-jt accumulation chains stop
                        # staggered and the output drain pipelines.
                        for jt in range(JT):
                            for ki, (k, w0, use01) in enumerate(K_PLAN):
                                rhs = at[:, 0:2] if use01 else at[:, 2:4]
                                nc.tensor.matmul(
                                    ps[jt][:, :],
                                    lhsT=ctj[:, w0:w0 + 2, jt],
                                    rhs=rhs,
                                    start=False,
                                    stop=(ki == len(K_PLAN) - 1),
                                    perf_mode=PM.DoubleRow,
                                )
                            ot = outp.tile([128, NB], F32, name=f"ot{jt}",
                                           tag="ot")
                            if jt % 2 == 0:
                                nc.scalar.activation(ot, ps[jt], AF.Copy)
                            else:
                                nc.vector.tensor_copy(ot, ps[jt])
                            nc.sync.dma_start(
                                yt_d[jt * 128:(jt + 1) * 128, :], ot)

    nc.compile()
    return nc


def _moments():
    """E[A_i * A_j | seg] and E[A_i * t^p | seg] for the lo-slot projection,
    with A0 = mask, A1 = Q8(mask * t), under x ~ N(0,1) clipped to [-1,1].
    Monte-Carlo with a fixed seed (prep-time approximation choice only)."""
    rng = np.random.default_rng(12345)
    xs = rng.standard_normal(4_000_000).astype(np.float32)
    xc = np.clip(xs, -1, 1).astype(np.float32)
    seg = np.rint(xc * np.float32(4.5) + np.float32(4.0)).astype(np.int32)
    t = (xc * np.float32(4.5) + np.float32(4.5) - seg).astype(np.float32)
    t8 = t.astype(NP8).astype(np.float32)
    t2f = (t * t).astype(np.float32)
    a2 = t2f.astype(NP8).astype(np.float32)
    a3 = (t2f * t).astype(np.float32).astype(NP8).astype(np.float32)
    G = np.zeros((S, 2, 2))
    Bt = np.zeros((S, 2, P))     # E[A_i t^p]
    Ba = np.zeros((S, 2, P))     # E[A_i A_p]
    for s in range(S):
        m = seg == s
        one = np.ones(m.sum(), np.float32)
        A = [one, t8[m]]
        Ap = [one, t8[m], a2[m], a3[m]]
        tp = [one, t[m], t2f[m], (t2f * t)[m]]
        for i in range(2):
            for j in range(2):
                G[s, i, j] = np.mean(A[i] * A[j])
            for p in range(P):
                Bt[s, i, p] = np.mean(A[i] * tp[p])
                Ba[s, i, p] = np.mean(A[i] * Ap[p])
    return G, Bt, Ba


def _pack_coeff(coeff, scale):
    """coeff [OUT, IN, S, P] f32 -> [ICH*S, 128, NSLOT*JT*128] fp8:
    4 hi slots Q8(c*scale) + 2 lo slots carrying the LS projection of the
    quantization residual onto span{A0, A1}."""
    cs = coeff * scale[:, None, None, None]
    c_hi = cs.astype(NP8)
    c_hif = c_hi.astype(np.float32)

    G, Bt, Ba = _moments()
    lam = np.zeros((OUT, IN, S, 2), np.float32)
    for s in range(S):
        b0 = sum(cs[:, :, s, p] * Bt[s, 0, p] - c_hif[:, :, s, p] * Ba[s, 0, p]
                 for p in range(P))
        b1 = sum(cs[:, :, s, p] * Bt[s, 1, p] - c_hif[:, :, s, p] * Ba[s, 1, p]
                 for p in range(P))
        Gi = np.linalg.inv(G[s])
        lam[:, :, s, 0] = Gi[0, 0] * b0 + Gi[0, 1] * b1
        lam[:, :, s, 1] = Gi[1, 0] * b0 + Gi[1, 1] * b1
    c_lo = lam.astype(NP8)

    c2 = np.concatenate([c_hi, c_lo], axis=-1)            # [OUT, IN, S, 6]
    c2 = c2.reshape(JT, 128, ICH, 128, S, NSLOT)          # jt jin ic iin s k
    c2 = c2.transpose(2, 4, 3, 5, 0, 1)                   # ic s iin k jt jin
    return np.ascontiguousarray(c2.reshape(ICH * S, 128, NSLOT * JT * 128))


def kernel(x, coeff, scale, _trace=False):
    global LAST_EXEC_NS, LAST_RESULTS, LAST_NC, LAST_IN_MAPS
    x = np.ascontiguousarray(np.asarray(x, dtype=np.float32))
    coeff = np.ascontiguousarray(np.asarray(coeff, dtype=np.float32))
    scale = np.ascontiguousarray(np.asarray(scale, dtype=np.float32))

    xt = np.ascontiguousarray(x.T)                        # [IN, B]
    cf = _pack_coeff(coeff, scale)

    nc = _build_nc()
    in_maps = [
        {"xt": np.ascontiguousarray(xt[:, g * NB:(g + 1) * NB]),
         "coeff8": cf}
        for g in range(NC)
    ]
    res = run_bass_kernel_spmd(nc, in_maps, core_ids=list(range(NC)),
                               trace=_trace)
    LAST_RESULTS = res
    LAST_EXEC_NS = res.exec_time_ns
    LAST_NC = nc
    LAST_IN_MAPS = in_maps

    yt = np.concatenate([res.results[g]["yt"] for g in range(NC)], axis=1)
    return np.ascontiguousarray(yt.T)


# revision 30
# speedup vs baseline: 2.4134x; 1.0247x over previous
"""Trainium2 Bass kernel for a KAN layer (piecewise-cubic spline edges).

y[b, j] = scale[j] * sum_i sum_p coeff[j, i, seg(x[b,i]), p] * t(x[b,i])^p

with 9 uniform segments on [-1, 1], t the within-segment coordinate.

Strategy (fp8e4m3 + DoubleRow, 6 coefficient slots):
  * One-hot-masked GEMM: all matmul operands are fp8e4m3 and every matmul
    uses MatmulPerfMode.DoubleRow (256 contraction rows/instruction at 0.5
    cycles per output element).  3 DoubleRow matmuls per (segment, ichunk,
    jtile): hi(p0,p1), hi(p2,p3), lo(l0,l1).
  * Coefficient slots: hi = Q8(c*scale) for all 4 powers; the lo pair rides
    the (mask, mask*t) moving tiles and carries the least-squares projection
    of the full quantization residual onto span{1, t} (per-segment moments
    of the clipped-normal t-distribution) -> rel err ~1.2e-2, same as an
    8-slot hi/lo split, at 3/4 the PE work.
  * Moving tiles via byte tricks: seg as int8; m01 = (seg==s) per byte;
    mFF = m01 * 255 (int32 view; (m<<8)-m has no cross-byte borrow);
    at0/at1/at3 = bitwise-AND selects of fp8 t-power bytes (DVE int32 ops,
    2x mode); at2 = ACT Square(at1) since (m*t)^2 = m*t^2.  GPSIMD covers
    t^3 products.  All engines stay under the PE's 46us of matmul work.
  * 8-way data parallel over batch (each core: 512 batch cols, full OUT).
"""

import numpy as np
import ml_dtypes

import concourse.bass as bass
import concourse.mybir as mybir
from concourse import bacc
from concourse.tile import TileContext
from concourse.bass_utils import run_bass_kernel_spmd

AF = mybir.ActivationFunctionType
OP = mybir.AluOpType
PM = mybir.MatmulPerfMode
F32 = mybir.dt.float32
FP8 = mybir.dt.float8e4
I32 = mybir.dt.int32
I8 = mybir.dt.int8
NP8 = ml_dtypes.float8_e4m3

B, IN, OUT = 4096, 512, 512
S, P = 9, 4            # segments, polynomial terms
NC = 8                 # cores
NB = B // NC           # local batch (moving free dim)
ICH = IN // 128        # input chunks (contraction tiles)
JT = OUT // 128        # output-row tiles
NSLOT = 6              # coeff pair slots: hi p0..p3, lo l0..l1

# Tunables
AT_BUFS = 8            # in-flight masked-power tile groups
CT_BUFS = 8            # in-flight coeff tile groups
N_WARM = 30            # PE p-state warm-up dummy matmuls before the stream
# (k, weight-slot pair, use at[0:2]) order: at01-consumers first
K_PLAN = ((0, 0, True), (2, 4, True), (1, 2, False))

LAST_EXEC_NS = None
LAST_RESULTS = None
LAST_NC = None
LAST_IN_MAPS = None


def _build_nc():
    nc = bacc.Bacc("TRN2", target_bir_lowering=False, debug=False, num_devices=NC)

    xt_d = nc.dram_tensor("xt", [IN, NB], F32, kind="ExternalInput")
    cf_d = nc.dram_tensor("coeff8", [ICH * S, 128, NSLOT * JT * 128], FP8,
                          kind="ExternalInput")
    yt_d = nc.dram_tensor("yt", [OUT, NB], F32, kind="ExternalOutput")

    with TileContext(nc) as tc:
        with (
            tc.tile_pool(name="xp", bufs=1) as xp,
            tc.tile_pool(name="mp", bufs=4) as mp,
            tc.tile_pool(name="atp", bufs=AT_BUFS) as atp,
            tc.tile_pool(name="ctp", bufs=CT_BUFS) as ctp,
            tc.tile_pool(name="outp", bufs=JT) as outp,
            tc.tile_pool(name="pp", bufs=1, space="PSUM") as pp,
        ):
            xt_sb = xp.tile([128, ICH, NB], F32, name="xt_sb")
            xt_r = xt_d.rearrange("(c p) b -> p c b", p=128)

            # PE p-state warm-up: harmless DoubleRow matmuls on a zeroed tile
            # into a scratch PSUM bank keep the tensor engine continuously
            # busy from ~1.4us so the real stream starts at full clock.
            zt = xp.tile([128, 2, NB], FP8, name="zt")
            psd = pp.tile([128, NB], F32, name="psd", tag="psd")
            nc.vector.memset(zt, 0.0)
            for _ in range(N_WARM):
                nc.tensor.matmul(psd, lhsT=zt[:, :, 0:128], rhs=zt,
                                 start=True, stop=True, perf_mode=PM.DoubleRow)

            # x for ic0 first, then the first coeff tile, then the rest of x,
            # so the head-of-pipe dependencies resolve earliest.
            nc.sync.dma_start(xt_sb[:, 0], xt_r[:, 0])
            ct0 = ctp.tile([128, NSLOT, JT * 128], FP8, name="ct_0_0", tag="ct")
            nc.sync.dma_start(ct0, cf_d[0].rearrange("p (k q) -> p k q", k=NSLOT))
            for ic in range(1, ICH):
                nc.sync.dma_start(xt_sb[:, ic], xt_r[:, ic])

            xc_sb = xp.tile([128, ICH, NB], F32, name="xc_sb")
            t_sb = xp.tile([128, ICH, NB], F32, name="t_sb")
            t2_sb = xp.tile([128, ICH, NB], F32, name="t2_sb")
            t3_sb = xp.tile([128, ICH, NB], F32, name="t3_sb")
            seg_sb = xp.tile([128, ICH, NB], F32, name="seg_sb")  # seg - 4.5
            segb_sb = xp.tile([128, ICH, NB], I8, name="segb_sb")
            t8_sb = xp.tile([128, ICH, NB], FP8, name="t8_sb")
            t28_sb = xp.tile([128, ICH, NB], FP8, name="t28_sb")
            t38_sb = xp.tile([128, ICH, NB], FP8, name="t38_sb")

            ps = [pp.tile([128, NB], F32, name=f"ps{jt}", tag=f"ps{jt}")
                  for jt in range(JT)]

            # one-group software pipeline: each group's at2/at3 build and its
            # k1 matmuls are emitted AFTER the next group's mask ops, so the
            # in-order DVE queue never parks on the ACT-made t^2 powers and
            # the PE always has ready matmuls.
            deferred = None

            def emit_at23(d):
                ic, s, at, ctj, ts_, t2s_, t3s_, ss_ = d
                last = (ic == ICH - 1 and s >= S - 2)
                if ic == 0 and s <= 1:
                    nc.vector.tensor_tensor(at[:, 3], at[:, 1], t2s_,
                                            OP.mult)
                    nc.vector.scalar_tensor_tensor(
                        at[:, 2], ss_, float(s) - 4.5, t2s_,
                        OP.is_equal, OP.mult)
                else:
                    nc.vector.tensor_tensor(at[:, 2].bitcast(I32),
                                            t28_sb[:, ic].bitcast(I32),
                                            mff_of[(ic, s)].bitcast(I32),
                                            OP.bitwise_and)
                    if s % 3 == 0 or last:
                        nc.vector.tensor_tensor(
                            at[:, 3].bitcast(I32),
                            t38_sb[:, ic].bitcast(I32),
                            mff_of[(ic, s)].bitcast(I32), OP.bitwise_and)
                    else:
                        nc.gpsimd.tensor_tensor(at[:, 3], at[:, 0], t3s_,
                                                OP.mult)

            def emit_k1(d, is_last):
                ic, s, at, ctj = d[0], d[1], d[2], d[3]
                k, w0, use01 = K_PLAN[2]
                rhs = at[:, 2:4]
                for jt in range(JT):
                    nc.tensor.matmul(
                        ps[jt][:, :],
                        lhsT=ctj[:, w0:w0 + 2, jt],
                        rhs=rhs,
                        start=False,
                        stop=is_last,
                        perf_mode=PM.DoubleRow,
                    )
                    if is_last:
                        ot = outp.tile([128, NB], F32, name=f"ot{jt}",
                                       tag="ot")
                        if jt % 2 == 0:
                            nc.scalar.activation(ot, ps[jt], AF.Copy)
                        else:
                            nc.vector.tensor_copy(ot, ps[jt])
                        nc.sync.dma_start(
                            yt_d[jt * 128:(jt + 1) * 128, :], ot)

            mff_of = {}

            for ic in range(ICH):
                xs = xt_sb[:, ic]
                xcs = xc_sb[:, ic]
                ts = t_sb[:, ic]
                t2s = t2_sb[:, ic]
                t3s = t3_sb[:, ic]
                ss = seg_sb[:, ic]
                sgb = segb_sb[:, ic]
                # xc = clip(x,-1,1); seg byte = RNE(xc*4.5+4.0) (== floor of
                # the segment coordinate, verified exact vs searchsorted);
                # ss = seg - 4.5; t = 4.5*xc - ss
                nc.vector.tensor_scalar(xcs, xs, 1.0, -1.0, OP.min, OP.max)
                nc.vector.tensor_scalar(sgb, xcs, 4.5, 4.0, OP.mult, OP.add)
                nc.vector.tensor_scalar(ss, sgb, -4.5, None, OP.add)
                nc.vector.scalar_tensor_tensor(ts, xcs, 4.5, ss,
                                               OP.mult, OP.subtract)
                # fp8 t powers for AND-selects; f32 t^2, t^3 for Pool products.
                # t8 via DVE copy (shorter head path); ACT leads with Square
                # so the deferred at2/at3 deps resolve early.
                nc.scalar.activation(t2s, ts, AF.Square)
                nc.vector.tensor_copy(t8_sb[:, ic], ts)
                nc.scalar.activation(t28_sb[:, ic], t2s, AF.Copy)
                nc.gpsimd.tensor_tensor(t3s, t2s, ts, OP.mult)
                nc.scalar.activation(t38_sb[:, ic], t3s, AF.Copy)

                for s in range(S):
                    at = atp.tile([128, P, NB], FP8, name=f"at_{ic}_{s}", tag="at")
                    if ic == 0 and s == 0:
                        ct = ct0
                    else:
                        ct = ctp.tile([128, NSLOT, JT * 128], FP8,
                                      name=f"ct_{ic}_{s}", tag="ct")
                        nc.sync.dma_start(ct, cf_d[ic * S + s].rearrange(
                            "p (k q) -> p k q", k=NSLOT))
                    # byte mask 0xFF where seg==s in ONE dual op, then
                    # bitwise AND-selects of the fp8 t-power bytes (int32
                    # views, DVE 2x path)
                    mff = mp.tile([128, NB], I8, name=f"mff_{ic}_{s}",
                                  tag="mff")
                    mff_of[(ic, s)] = mff
                    nc.vector.tensor_scalar(mff, sgb, s, -1,
                                            OP.is_equal, OP.mult)
                    nc.vector.tensor_scalar(at[:, 0].bitcast(I32),
                                            mff.bitcast(I32),
                                            0x38383838, None, OP.bitwise_and)
                    nc.vector.tensor_tensor(at[:, 1].bitcast(I32),
                                            t8_sb[:, ic].bitcast(I32),
                                            mff.bitcast(I32), OP.bitwise_and)

                    first = (ic == 0 and s == 0)
                    last = (ic == ICH - 1 and s == S - 1)
                    ctj = ct.rearrange("p k (j q) -> p k j q", j=JT)
                    cur = (ic, s, at, ctj, ts, t2s, t3s, ss)
                    if not last:
                        for k, w0, use01 in K_PLAN[:2]:
                            for jt in range(JT):
                                nc.tensor.matmul(
                                    ps[jt][:, :],
                                    lhsT=ctj[:, w0:w0 + 2, jt],
                                    rhs=at[:, 0:2],
                                    start=(first and k == 0),
                                    stop=False,
                                    perf_mode=PM.DoubleRow,
                                )
                        if deferred is not None:
                            emit_at23(deferred)
                            emit_k1(deferred, False)
                        deferred = cur
                    else:
                        # drain: build the last group's at tiles up front, then
                        # jt-major over the final 4 matmuls of each chain so
                        # the stops stagger 428ns and copies/DMAs pipeline
                        emit_at23(cur)
                        d34 = deferred
                        emit_at23(d34)
                        at34, ctj34 = d34[2], d34[3]
                        for jt in range(JT):
                            nc.tensor.matmul(
                                ps[jt][:, :],
                                lhsT=ctj34[:, K_PLAN[2][1]:K_PLAN[2][1] + 2, jt],
                                rhs=at34[:, 2:4], start=False, stop=False,
                                perf_mode=PM.DoubleRow)
                            for k, w0, use01 in K_PLAN:
                                rhs = at[:, 0:2] if use01 else at[:, 2:4]
                                nc.tensor.matmul(
                                    ps[jt][:, :],
                                    lhsT=ctj[:, w0:w0 + 2, jt],
                                    rhs=rhs, start=False,
                                    stop=(k == K_PLAN[2][0]),
                                    perf_mode=PM.DoubleRow)
                            ot = outp.tile([128, NB], F32, name=f"ot{jt}",
                                           tag="ot")
                            if jt % 2 == 0:
                                nc.scalar.activation(ot, ps[jt], AF.Copy)
                            else:
                                nc.vector.tensor_copy(ot, ps[jt])
                            nc.sync.dma_start(
                                yt_d[jt * 128:(jt + 1) * 128, :], ot)

    nc.compile()
    return nc


def _moments():
    """E[A_i * A_j | seg] and E[A_i * t^p | seg] for the lo-slot projection,
    with A0 = mask, A1 = Q8(mask * t), under x ~ N(0,1) clipped to [-1,1].
    Monte-Carlo with a fixed seed (prep-time approximation choice only)."""
    rng = np.random.default_rng(12345)
    xs = rng.standard_normal(4_000_000).astype(np.float32)
    xc = np.clip(xs, -1, 1).astype(np.float32)
    seg = np.rint(xc * np.float32(4.5) + np.float32(4.0)).astype(np.int32)
    t = (xc * np.float32(4.5) + np.float32(4.5) - seg).astype(np.float32)
    t8 = t.astype(NP8).astype(np.float32)
    t2f = (t * t).astype(np.float32)
    a2 = t2f.astype(NP8).astype(np.float32)
    a3 = (t2f * t).astype(np.float32).astype(NP8).astype(np.float32)
    G = np.zeros((S, 2, 2))
    Bt = np.zeros((S, 2, P))     # E[A_i t^p]
    Ba = np.zeros((S, 2, P))     # E[A_i A_p]
    for s in range(S):
        m = seg == s
        one = np.ones(m.sum(), np.float32)
        A = [one, t8[m]]
        Ap = [one, t8[m], a2[m], a3[m]]
        tp = [one, t[m], t2f[m], (t2f * t)[m]]
        for i in range(2):
            for j in range(2):
                G[s, i, j] = np.mean(A[i] * A[j])
            for p in range(P):
                Bt[s, i, p] = np.mean(A[i] * tp[p])
                Ba[s, i, p] = np.mean(A[i] * Ap[p])
    return G, Bt, Ba


def _pack_coeff(coeff, scale):
    """coeff [OUT, IN, S, P] f32 -> [ICH*S, 128, NSLOT*JT*128] fp8:
    4 hi slots Q8(c*scale) + 2 lo slots carrying the LS projection of the
    quantization residual onto span{A0, A1}."""
    cs = coeff * scale[:, None, None, None]
    c_hi = cs.astype(NP8)
    c_hif = c_hi.astype(np.float32)

    G, Bt, Ba = _moments()
    lam = np.zeros((OUT, IN, S, 2), np.float32)
    for s in range(S):
        b0 = sum(cs[:, :, s, p] * Bt[s, 0, p] - c_hif[:, :, s, p] * Ba[s, 0, p]
                 for p in range(P))
        b1 = sum(cs[:, :, s, p] * Bt[s, 1, p] - c_hif[:, :, s, p] * Ba[s, 1, p]
                 for p in range(P))
        Gi = np.linalg.inv(G[s])
        lam[:, :, s, 0] = Gi[0, 0] * b0 + Gi[0, 1] * b1
        lam[:, :, s, 1] = Gi[1, 0] * b0 + Gi[1, 1] * b1
    c_lo = lam.astype(NP8)

    c2 = np.concatenate([c_hi, c_lo], axis=-1)            # [OUT, IN, S, 6]
    c2 = c2.reshape(JT, 128, ICH, 128, S, NSLOT)          # jt jin ic iin s k
    c2 = c2.transpose(2, 4, 3, 5, 0, 1)                   # ic s iin k jt jin
    return np.ascontiguousarray(c2.reshape(ICH * S, 128, NSLOT * JT * 128))


def kernel(x, coeff, scale, _trace=False):
    global LAST_EXEC_NS, LAST_RESULTS, LAST_NC, LAST_IN_MAPS
    x = np.ascontiguousarray(np.asarray(x, dtype=np.float32))
    coeff = np.ascontiguousarray(np.asarray(coeff, dtype=np.float32))
    scale = np.ascontiguousarray(np.asarray(scale, dtype=np.float32))

    xt = np.ascontiguousarray(x.T)                        # [IN, B]
    cf = _pack_coeff(coeff, scale)

    nc = _build_nc()
    in_maps = [
        {"xt": np.ascontiguousarray(xt[:, g * NB:(g + 1) * NB]),
         "coeff8": cf}
        for g in range(NC)
    ]
    res = run_bass_kernel_spmd(nc, in_maps, core_ids=list(range(NC)),
                               trace=_trace)
    LAST_RESULTS = res
    LAST_EXEC_NS = res.exec_time_ns
    LAST_NC = nc
    LAST_IN_MAPS = in_maps

    yt = np.concatenate([res.results[g]["yt"] for g in range(NC)], axis=1)
    return np.ascontiguousarray(yt.T)
